# revision 6
# baseline (speedup 1.0000x reference)
"""Distributed volume-argmin (correlation volume max/argmax) on 8 NeuronCores.

Problem: feat_l, feat_r [1, 128, 96, 128] fp32.
  corr[b,h,w,ij] = sum_c feat_l[b,c,h,w] * feat_r[b,c,i,j]
  flow_cost = max over ij, flat = argmax over ij,
  flow = (xoff - (flat % W_R)*scale_x, yoff - (flat // W_R)*scale_y)

Sharding: the 96 left-image rows are split across 8 cores (12 rows = 1536
left pixels each); feat_r is replicated. Each core computes its
[1536 x 12288] correlation block (K=C=128 contraction on partitions) and a
full per-pixel max/argmax, so no cross-core reduction is needed — outputs
concatenate.

Matmul precision: inputs are split host-side into bf16 hi/lo pairs and each
512-chunk accumulates hi*hi + hi*lo + lo*hi in fp32 PSUM (fp32-grade
accuracy at 3x-bf16-pass cost).

Argmax strategy per 128-pixel tile (partition = left pixel, free = 12288
right pixels): a binary fold tree on the Vector engine
  colmax1[j] = max(x[j], x[j+6144])        (reads PSUM directly, drains it)
  colmax2[j] = max(colmax1[j], colmax1[j+3072])
  colmax3[j] = max(colmax2[j], colmax2[j+1536])
then MAX8 + FIND_INDEX8 on just 1536 elements give the exact max M and the
fold-residue index i*. The three fold bits are recovered exactly on the
Scalar engine: count_k = sum_j sign(M - upper_half_k[j]) equals the half
size minus 1 exactly when M lives in that upper half (sign(0)=0 on HW,
verified). flat = i* + 1536*c + 3072*b + 6144*a. The upper input half is
evicted PSUM->SBUF by ScalarE (needed as a fold operand and for count_1);
chunk groups are produced upper-half-first so every PSUM bank is drained
promptly.
"""

import sys

for p in ("/opt/trn_rl_repo",):
    if p not in sys.path:
        sys.path.insert(0, p)

import numpy as np
import ml_dtypes

import concourse.bass as bass
import concourse.tile as tile
from concourse import bacc, mybir
from concourse.bass_utils import run_bass_kernel_spmd

# Problem geometry (hardcoded per the task contract).
B, C, H, W = 1, 128, 96, 128
H_R, W_R = 96, 128
NPIX = H_R * W_R              # 12288 right pixels
NCORES = 8
ROWS_PER_CORE = H // NCORES   # 12 left rows per core
NT = ROWS_PER_CORE            # one tile per left row (128 pixels)
CHUNK = 512
GRP = 2048                    # psum group (4 banks)
NGRP = NPIX // GRP            # 6
W1, W2, W3 = NPIX // 2, NPIX // 4, NPIX // 8   # 6144, 3072, 1536

_F32 = mybir.dt.float32
_BF16 = mybir.dt.bfloat16
_U32 = mybir.dt.uint32
_FP8 = mybir.dt.float8e4

_compiled = {}


def _emit_group_mms(nc, ps_ap, lh, ll, rhs_hi, rhs_lo, g):
    """3-pass bf16 matmuls for the 4 chunks of group g into psum tile ps.

    Pass-major order (hi*hi x4, hi*lo x4, lo*hi x4) so consecutive matmuls
    write different PSUM banks — same-bank accumulation back-to-back
    serializes the PE's fill/drain pipeline (measured 379 ns vs 216 ns)."""
    for pass_lhs, pass_rhs, st, sp in (
        (lh, rhs_hi, True, False),
        (lh, rhs_lo, False, False),
        (ll, rhs_hi, False, True),
    ):
        for s in range(4):
            col0 = g * GRP + s * CHUNK
            out = ps_ap[:, s * CHUNK:(s + 1) * CHUNK]
            nc.tensor.matmul(out, pass_lhs, pass_rhs[:, col0:col0 + CHUNK],
                             start=st, stop=sp)


def _build_nc():
    nc = bacc.Bacc("TRN2", target_bir_lowering=False, debug=False,
                   num_devices=NCORES)

    lhs_hi_d = nc.declare_dram_parameter("lhs_hi", [C, NT * 128], _BF16,
                                         isOutput=False)
    lhs_lo_d = nc.declare_dram_parameter("lhs_lo", [C, NT * 128], _BF16,
                                         isOutput=False)
    rhs_hi_d = nc.declare_dram_parameter("rhs_hi", [C, NPIX], _BF16,
                                         isOutput=False)
    rhs_lo_d = nc.declare_dram_parameter("rhs_lo", [C, NPIX], _BF16,
                                         isOutput=False)
    cost_d = nc.declare_dram_parameter("cost", [128, NT], _F32, isOutput=True)
    idx_d = nc.declare_dram_parameter("idx", [128, NT], _U32, isOutput=True)
    cnt1_d = nc.declare_dram_parameter("cnt1", [128, NT], _F32, isOutput=True)
    cnt2_d = nc.declare_dram_parameter("cnt2", [128, NT], _F32, isOutput=True)
    cnt3_d = nc.declare_dram_parameter("cnt3", [128, NT], _F32, isOutput=True)

    with tile.TileContext(nc) as tc:
        with (
            tc.tile_pool(name="rhs", bufs=1) as rhs_pool,
            tc.tile_pool(name="lhs", bufs=1) as lhs_pool,
            tc.tile_pool(name="x2p", bufs=2) as x2_pool,
            tc.tile_pool(name="cm1p", bufs=2) as cm1_pool,
            tc.tile_pool(name="cm2p", bufs=1) as cm2_pool,
            tc.tile_pool(name="cm3p", bufs=2) as cm3_pool,
            tc.tile_pool(name="dumpp", bufs=1) as dump_pool,
            tc.tile_pool(name="m8p", bufs=3) as m8_pool,
            tc.tile_pool(name="ps", bufs=2, space="PSUM") as ps_pool,
            tc.tile_pool(name="outcols", bufs=1) as out_pool,
        ):
            lhs_hi = lhs_pool.tile([C, NT * 128], _BF16, tag="lh")
            lhs_lo = lhs_pool.tile([C, NT * 128], _BF16, tag="ll")
            nc.sync.dma_start(lhs_hi[:], lhs_hi_d[:])
            nc.sync.dma_start(lhs_lo[:], lhs_lo_d[:])

            # rhs loaded in 4 column groups so tile-0 matmuls can start early
            rhs_hi = rhs_pool.tile([C, NPIX], _BF16, tag="rh")
            rhs_lo = rhs_pool.tile([C, NPIX], _BF16, tag="rl")
            LW = NPIX // 4
            for q in (2, 3, 0, 1):
                sl = slice(q * LW, (q + 1) * LW)
                nc.sync.dma_start(rhs_hi[:, sl], rhs_hi_d[:, sl])
                nc.sync.dma_start(rhs_lo[:, sl], rhs_lo_d[:, sl])

            cost_cols = out_pool.tile([128, NT], _F32, tag="cc")
            idx_cols = out_pool.tile([128, NT], _U32, tag="ic")
            cnt1_cols = out_pool.tile([128, NT], _F32, tag="c1")
            cnt2_cols = out_pool.tile([128, NT], _F32, tag="c2")
            cnt3_cols = out_pool.tile([128, NT], _F32, tag="c3")

            dump = dump_pool.tile([128, W1], _FP8, tag="dump")

            prev = None  # (x2, cm1, cm2, m8, t) of previous tile

            def emit_counts(state):
                # cnt2 first: the next tile's folds WAR-wait on cm1 (bufs=1),
                # so clear its reader as early as possible; big cnt1 last.
                x2_p, cm1_p, cm2_p, m8_p, tp = state
                nc.scalar.activation(
                    dump[:, 0:W2], cm1_p[:, W2:W1],
                    mybir.ActivationFunctionType.Sign,
                    bias=m8_p[:, 0:1], scale=-1.0,
                    accum_out=cnt2_cols[:, tp:tp + 1])
                nc.scalar.activation(
                    dump[:, 0:W3], cm2_p[:, W3:W2],
                    mybir.ActivationFunctionType.Sign,
                    bias=m8_p[:, 0:1], scale=-1.0,
                    accum_out=cnt3_cols[:, tp:tp + 1])
                nc.scalar.activation(
                    dump[:, 0:W1], x2_p[:],
                    mybir.ActivationFunctionType.Sign,
                    bias=m8_p[:, 0:1], scale=-1.0,
                    accum_out=cnt1_cols[:, tp:tp + 1])

            for t in range(NT):
                lh = lhs_hi[:, t * 128:(t + 1) * 128]
                ll = lhs_lo[:, t * 128:(t + 1) * 128]

                x2 = x2_pool.tile([128, W1], _F32, tag="x2")
                cm1 = cm1_pool.tile([128, W1], _F32, tag="cm1")
                # interleave upper (evict) and lower (fold) groups so PSUM
                # slots alternate between the fast ScalarE drain and the DVE
                # fold drain, and the fold chain starts early: fold1_g needs
                # only evict(g+3) and psum g.
                for g in (3, 0, 4, 1, 5, 2):
                    ps = ps_pool.tile([128, GRP], _F32, tag="ps")
                    _emit_group_mms(nc, ps[:], lh, ll, rhs_hi[:], rhs_lo[:], g)
                    if g >= 3:
                        nc.scalar.copy(x2[:, (g - 3) * GRP:(g - 2) * GRP],
                                       ps[:])
                        if g == 5 and prev is not None:
                            # previous tile's fold-bit counts: after this
                            # tile's last evict so ScalarE keeps PSUM drained
                            emit_counts(prev)
                    else:
                        sl = slice(g * GRP, (g + 1) * GRP)
                        nc.vector.tensor_max(cm1[:, sl], ps[:], x2[:, sl])

                cm2 = cm2_pool.tile([128, W2], _F32, tag="cm2")
                nc.vector.tensor_max(cm2[:], cm1[:, 0:W2], cm1[:, W2:W1])
                cm3 = cm3_pool.tile([128, W3], _F32, tag="cm3")
                nc.vector.tensor_max(cm3[:], cm2[:, 0:W3], cm2[:, W3:W2])

                m8 = m8_pool.tile([128, 8], _F32, tag="m8")
                i8 = m8_pool.tile([128, 8], _U32, tag="i8")
                nc.vector.max(m8[:], cm3[:])
                nc.vector.max_index(i8[:], m8[:], cm3[:])
                nc.vector.tensor_copy(cost_cols[:, t:t + 1], m8[:, 0:1])
                nc.vector.tensor_copy(idx_cols[:, t:t + 1], i8[:, 0:1])

                prev = (x2, cm1, cm2, m8, t)

            emit_counts(prev)

            nc.sync.dma_start(cost_d[:], cost_cols[:])
            nc.sync.dma_start(idx_d[:], idx_cols[:])
            nc.sync.dma_start(cnt1_d[:], cnt1_cols[:])
            nc.sync.dma_start(cnt2_d[:], cnt2_cols[:])
            nc.sync.dma_start(cnt3_d[:], cnt3_cols[:])

    nc.finalize()
    return nc


def _get_nc():
    if "nc" not in _compiled:
        _compiled["nc"] = _build_nc()
    return _compiled["nc"]


def _split_bf16(x):
    hi = x.astype(ml_dtypes.bfloat16)
    lo = (x - hi.astype(np.float32)).astype(ml_dtypes.bfloat16)
    return hi, lo


def _make_in_maps(feat_l, feat_r):
    rhs = np.ascontiguousarray(feat_r.reshape(C, NPIX))
    rhs_hi, rhs_lo = _split_bf16(rhs)
    in_maps = []
    for k in range(NCORES):
        lhs = np.ascontiguousarray(
            feat_l[0, :, k * ROWS_PER_CORE:(k + 1) * ROWS_PER_CORE, :]
            .reshape(C, ROWS_PER_CORE * W))
        lhs_hi, lhs_lo = _split_bf16(lhs)
        in_maps.append({
            "lhs_hi": lhs_hi, "lhs_lo": lhs_lo,
            "rhs_hi": rhs_hi, "rhs_lo": rhs_lo,
        })
    return in_maps


def _decode(results, scale_x, scale_y):
    flow_cost = np.empty((B, H, W), np.float32)
    flat = np.empty((H, W), np.int64)
    for k, r in enumerate(results):
        rows = slice(k * ROWS_PER_CORE, (k + 1) * ROWS_PER_CORE)
        flow_cost[0, rows, :] = r["cost"].T
        a = (W1 - r["cnt1"].T.astype(np.int64))
        b = (W2 - r["cnt2"].T.astype(np.int64))
        c = (W3 - r["cnt3"].T.astype(np.int64))
        i_res = r["idx"].T.astype(np.int64)
        flat[rows, :] = i_res + W3 * c + W2 * b + W1 * a
    assert flat.min() >= 0 and flat.max() < NPIX, (flat.min(), flat.max())
    xoff = np.arange(W)
    yoff = np.arange(H)
    u = -((flat % W_R) * scale_x - xoff[None, :]).astype(np.float32)
    v = -((flat // W_R) * scale_y - yoff[:, None]).astype(np.float32)
    flow = np.stack([u, v], axis=2)[None]  # [1, H, W, 2]
    return flow, flow_cost


def kernel(feat_l, feat_r, scale_x, scale_y):
    feat_l = np.asarray(feat_l, dtype=np.float32)
    feat_r = np.asarray(feat_r, dtype=np.float32)
    nc = _get_nc()
    in_maps = _make_in_maps(feat_l, feat_r)
    res = run_bass_kernel_spmd(nc, in_maps, core_ids=list(range(NCORES)))
    return _decode(res.results, int(scale_x), int(scale_y))


def run_timed(np_inputs, trace_cores=None):
    """Run once with NTFF tracing enabled; returns exec_time_ns (or None)."""
    feat_l = np.asarray(np_inputs["feat_l"], dtype=np.float32)
    feat_r = np.asarray(np_inputs["feat_r"], dtype=np.float32)
    nc = _get_nc()
    in_maps = _make_in_maps(feat_l, feat_r)
    res = run_bass_kernel_spmd(nc, in_maps, core_ids=list(range(NCORES)),
                               trace=True, trace_cores=trace_cores)
    if res.instructions_and_trace is not None:
        print("trace path:", res.instructions_and_trace[1])
    return res.exec_time_ns


# revision 9
# speedup vs baseline: 1.1013x; 1.1013x over previous
"""Distributed volume-argmin (correlation volume max/argmax) on 8 NeuronCores.

Problem: feat_l, feat_r [1, 128, 96, 128] fp32.
  corr[b,h,w,ij] = sum_c feat_l[b,c,h,w] * feat_r[b,c,i,j]
  flow_cost = max over ij, flat = argmax over ij,
  flow = (xoff - (flat % W_R)*scale_x, yoff - (flat // W_R)*scale_y)

Sharding: the 96 left-image rows are split across 8 cores (12 rows = 1536
left pixels each); feat_r is replicated. Each core computes its
[1536 x 12288] correlation block (K=C=128 contraction on partitions) and a
full per-pixel max/argmax, so no cross-core reduction is needed — outputs
concatenate.

Matmul precision: inputs are split host-side into bf16 hi/lo pairs and each
512-chunk accumulates hi*hi + hi*lo + lo*hi in fp32 PSUM (fp32-grade
accuracy at 3x-bf16-pass cost).

Argmax strategy per 128-pixel tile (partition = left pixel, free = 12288
right pixels): a binary fold tree on the Vector engine
  colmax1[j] = max(x[j], x[j+6144])        (reads PSUM directly, drains it)
  colmax2[j] = max(colmax1[j], colmax1[j+3072])
  colmax3[j] = max(colmax2[j], colmax2[j+1536])
then MAX8 + FIND_INDEX8 on just 1536 elements give the exact max M and the
fold-residue index i*. The three fold bits are recovered exactly on the
Scalar engine: count_k = sum_j sign(M - upper_half_k[j]) equals the half
size minus 1 exactly when M lives in that upper half (sign(0)=0 on HW,
verified). flat = i* + 1536*c + 3072*b + 6144*a. The upper input half is
evicted PSUM->SBUF by ScalarE (needed as a fold operand and for count_1);
chunk groups are produced upper-half-first so every PSUM bank is drained
promptly.
"""

import sys

for p in ("/opt/trn_rl_repo",):
    if p not in sys.path:
        sys.path.insert(0, p)

import numpy as np
import ml_dtypes

import concourse.bass as bass
import concourse.tile as tile
from concourse import bacc, mybir
from concourse.bass_utils import run_bass_kernel_spmd

# Problem geometry (hardcoded per the task contract).
B, C, H, W = 1, 128, 96, 128
H_R, W_R = 96, 128
NPIX = H_R * W_R              # 12288 right pixels
NCORES = 8
ROWS_PER_CORE = H // NCORES   # 12 left rows per core
NT = ROWS_PER_CORE            # one tile per left row (128 pixels)
CHUNK = 512
GRP = 2048                    # psum group (4 banks)
NGRP = NPIX // GRP            # 6
W1, W2, W3 = NPIX // 2, NPIX // 4, NPIX // 8   # 6144, 3072, 1536

_F32 = mybir.dt.float32
_BF16 = mybir.dt.bfloat16
_U32 = mybir.dt.uint32
_FP8 = mybir.dt.float8e4

_compiled = {}


def _emit_group_mms(nc, ps_ap, lh, ll, rhs_hi, rhs_lo, g):
    """3-pass bf16 matmuls for the 4 chunks of group g into psum tile ps.

    Pass-major order (hi*hi x4, hi*lo x4, lo*hi x4) so consecutive matmuls
    write different PSUM banks — same-bank accumulation back-to-back
    serializes the PE's fill/drain pipeline (measured 379 ns vs 216 ns)."""
    for pass_lhs, pass_rhs, st, sp in (
        (lh, rhs_hi, True, False),
        (lh, rhs_lo, False, False),
        (ll, rhs_hi, False, True),
    ):
        for s in range(4):
            col0 = g * GRP + s * CHUNK
            out = ps_ap[:, s * CHUNK:(s + 1) * CHUNK]
            nc.tensor.matmul(out, pass_lhs, pass_rhs[:, col0:col0 + CHUNK],
                             start=st, stop=sp)


def _build_nc():
    nc = bacc.Bacc("TRN2", target_bir_lowering=False, debug=False,
                   num_devices=NCORES)

    lhs_hi_d = nc.declare_dram_parameter("lhs_hi", [C, NT * 128], _BF16,
                                         isOutput=False)
    lhs_lo_d = nc.declare_dram_parameter("lhs_lo", [C, NT * 128], _BF16,
                                         isOutput=False)
    rhs_hi_d = nc.declare_dram_parameter("rhs_hi", [C, NPIX], _BF16,
                                         isOutput=False)
    rhs_lo_d = nc.declare_dram_parameter("rhs_lo", [C, NPIX], _BF16,
                                         isOutput=False)
    cost_d = nc.declare_dram_parameter("cost", [128, NT], _F32, isOutput=True)
    idx_d = nc.declare_dram_parameter("idx", [128, NT], _U32, isOutput=True)
    cnt1_d = nc.declare_dram_parameter("cnt1", [128, 3 * NT], _F32, isOutput=True)
    cnt2_d = nc.declare_dram_parameter("cnt2", [128, NT], _F32, isOutput=True)
    cnt3_d = nc.declare_dram_parameter("cnt3", [128, NT], _F32, isOutput=True)

    with tile.TileContext(nc) as tc:
        with (
            tc.tile_pool(name="rhs", bufs=1) as rhs_pool,
            tc.tile_pool(name="lhs", bufs=1) as lhs_pool,
            tc.tile_pool(name="x2p", bufs=2) as x2_pool,
            tc.tile_pool(name="cm1p", bufs=2) as cm1_pool,
            tc.tile_pool(name="cm2p", bufs=1) as cm2_pool,
            tc.tile_pool(name="cm3p", bufs=2) as cm3_pool,
            tc.tile_pool(name="dumpp", bufs=1) as dump_pool,
            tc.tile_pool(name="m8p", bufs=3) as m8_pool,
            tc.tile_pool(name="ps", bufs=2, space="PSUM") as ps_pool,
            tc.tile_pool(name="outcols", bufs=1) as out_pool,
        ):
            lhs_hi = lhs_pool.tile([C, NT * 128], _BF16, tag="lh")
            lhs_lo = lhs_pool.tile([C, NT * 128], _BF16, tag="ll")
            nc.sync.dma_start(lhs_hi[:], lhs_hi_d[:])
            nc.sync.dma_start(lhs_lo[:], lhs_lo_d[:])

            # rhs loaded in 4 column groups so tile-0 matmuls can start early
            rhs_hi = rhs_pool.tile([C, NPIX], _BF16, tag="rh")
            rhs_lo = rhs_pool.tile([C, NPIX], _BF16, tag="rl")
            for g in (3, 4, 5, 0, 1, 2):
                sl = slice(g * GRP, (g + 1) * GRP)
                nc.sync.dma_start(rhs_hi[:, sl], rhs_hi_d[:, sl])
                nc.sync.dma_start(rhs_lo[:, sl], rhs_lo_d[:, sl])

            cost_cols = out_pool.tile([128, NT], _F32, tag="cc")
            idx_cols = out_pool.tile([128, NT], _U32, tag="ic")
            cnt1_cols = out_pool.tile([128, 3 * NT], _F32, tag="c1")
            cnt2_cols = out_pool.tile([128, NT], _F32, tag="c2")
            cnt3_cols = out_pool.tile([128, NT], _F32, tag="c3")

            dump = dump_pool.tile([128, W1], _FP8, tag="dump")

            prev = None  # (x2, cm1, cm2, m8, t) of previous tile

            def emit_counts(state, on_dve=False):
                # cnt2/cnt3 first (they unblock cm1/cm2 reuse); the big cnt1
                # is split into 2048-wide pieces so the ScalarE queue never
                # holds a single long op when the next tile's evicts queue up.
                x2_p, cm1_p, cm2_p, m8_p, tp = state
                if on_dve:
                    # epilogue: DVE has slack after the last tile; ScalarE
                    # only does the cnt1 pieces.
                    nc.vector.tensor_tensor_reduce(
                        dump[:, 0:W2], cm1_p[:, W2:W1],
                        m8_p[:, 0:1].to_broadcast((128, W2)),
                        1.0, 0.0,
                        op0=mybir.AluOpType.is_lt, op1=mybir.AluOpType.add,
                        accum_out=cnt2_cols[:, tp:tp + 1])
                    nc.vector.tensor_tensor_reduce(
                        dump[:, 0:W3], cm2_p[:, W3:W2],
                        m8_p[:, 0:1].to_broadcast((128, W3)),
                        1.0, 0.0,
                        op0=mybir.AluOpType.is_lt, op1=mybir.AluOpType.add,
                        accum_out=cnt3_cols[:, tp:tp + 1])
                else:
                    nc.scalar.activation(
                        dump[:, 0:W2], cm1_p[:, W2:W1],
                        mybir.ActivationFunctionType.Sign,
                        bias=m8_p[:, 0:1], scale=-1.0,
                        accum_out=cnt2_cols[:, tp:tp + 1])
                    nc.scalar.activation(
                        dump[:, 0:W3], cm2_p[:, W3:W2],
                        mybir.ActivationFunctionType.Sign,
                        bias=m8_p[:, 0:1], scale=-1.0,
                        accum_out=cnt3_cols[:, tp:tp + 1])
                for piece in range(3):
                    sl = slice(piece * GRP, (piece + 1) * GRP)
                    nc.scalar.activation(
                        dump[:, sl], x2_p[:, sl],
                        mybir.ActivationFunctionType.Sign,
                        bias=m8_p[:, 0:1], scale=-1.0,
                        accum_out=cnt1_cols[:, 3 * tp + piece:3 * tp + piece + 1])

            for t in range(NT):
                lh = lhs_hi[:, t * 128:(t + 1) * 128]
                ll = lhs_lo[:, t * 128:(t + 1) * 128]

                x2 = x2_pool.tile([128, W1], _F32, tag="x2")
                cm1 = cm1_pool.tile([128, W1], _F32, tag="cm1")
                # upper half first: groups 3,4,5 evicted by ScalarE; then the
                # lower half 0,1,2 is fold-drained by DVE while ScalarE has a
                # free window for the previous tile's counts.
                for g in (3, 4, 5):
                    ps = ps_pool.tile([128, GRP], _F32, tag="ps")
                    _emit_group_mms(nc, ps[:], lh, ll, rhs_hi[:], rhs_lo[:], g)
                    nc.scalar.copy(x2[:, (g - 3) * GRP:(g - 2) * GRP], ps[:])

                if prev is not None:
                    emit_counts(prev)

                for g in (0, 1, 2):
                    ps = ps_pool.tile([128, GRP], _F32, tag="ps")
                    _emit_group_mms(nc, ps[:], lh, ll, rhs_hi[:], rhs_lo[:], g)
                    sl = slice(g * GRP, (g + 1) * GRP)
                    nc.vector.tensor_max(cm1[:, sl], ps[:], x2[:, sl])

                cm2 = cm2_pool.tile([128, W2], _F32, tag="cm2")
                nc.vector.tensor_max(cm2[:], cm1[:, 0:W2], cm1[:, W2:W1])
                cm3 = cm3_pool.tile([128, W3], _F32, tag="cm3")
                nc.vector.tensor_max(cm3[:], cm2[:, 0:W3], cm2[:, W3:W2])

                m8 = m8_pool.tile([128, 8], _F32, tag="m8")
                i8 = m8_pool.tile([128, 8], _U32, tag="i8")
                nc.vector.max(m8[:], cm3[:])
                nc.vector.max_index(i8[:], m8[:], cm3[:])
                nc.vector.tensor_copy(cost_cols[:, t:t + 1], m8[:, 0:1])
                nc.vector.tensor_copy(idx_cols[:, t:t + 1], i8[:, 0:1])

                prev = (x2, cm1, cm2, m8, t)

            emit_counts(prev, on_dve=False)

            nc.sync.dma_start(cost_d[:], cost_cols[:])
            nc.sync.dma_start(idx_d[:], idx_cols[:])
            nc.sync.dma_start(cnt1_d[:], cnt1_cols[:])
            nc.sync.dma_start(cnt2_d[:], cnt2_cols[:])
            nc.sync.dma_start(cnt3_d[:], cnt3_cols[:])

    nc.finalize()
    return nc


def _get_nc():
    if "nc" not in _compiled:
        _compiled["nc"] = _build_nc()
    return _compiled["nc"]


def _split_bf16(x):
    hi = x.astype(ml_dtypes.bfloat16)
    lo = (x - hi.astype(np.float32)).astype(ml_dtypes.bfloat16)
    return hi, lo


def _make_in_maps(feat_l, feat_r):
    rhs = np.ascontiguousarray(feat_r.reshape(C, NPIX))
    rhs_hi, rhs_lo = _split_bf16(rhs)
    in_maps = []
    for k in range(NCORES):
        lhs = np.ascontiguousarray(
            feat_l[0, :, k * ROWS_PER_CORE:(k + 1) * ROWS_PER_CORE, :]
            .reshape(C, ROWS_PER_CORE * W))
        lhs_hi, lhs_lo = _split_bf16(lhs)
        in_maps.append({
            "lhs_hi": lhs_hi, "lhs_lo": lhs_lo,
            "rhs_hi": rhs_hi, "rhs_lo": rhs_lo,
        })
    return in_maps


def _decode(results, scale_x, scale_y):
    flow_cost = np.empty((B, H, W), np.float32)
    flat = np.empty((H, W), np.int64)
    for k, r in enumerate(results):
        rows = slice(k * ROWS_PER_CORE, (k + 1) * ROWS_PER_CORE)
        flow_cost[0, rows, :] = r["cost"].T
        cnt1 = r["cnt1"].reshape(128, NT, 3).sum(axis=2)
        a = (W1 - cnt1.T.astype(np.int64))
        b = (W2 - r["cnt2"].T.astype(np.int64))
        c = (W3 - r["cnt3"].T.astype(np.int64))
        i_res = r["idx"].T.astype(np.int64)
        flat[rows, :] = i_res + W3 * c + W2 * b + W1 * a
    assert flat.min() >= 0 and flat.max() < NPIX, (flat.min(), flat.max())
    xoff = np.arange(W)
    yoff = np.arange(H)
    u = -((flat % W_R) * scale_x - xoff[None, :]).astype(np.float32)
    v = -((flat // W_R) * scale_y - yoff[:, None]).astype(np.float32)
    flow = np.stack([u, v], axis=2)[None]  # [1, H, W, 2]
    return flow, flow_cost


def kernel(feat_l, feat_r, scale_x, scale_y):
    feat_l = np.asarray(feat_l, dtype=np.float32)
    feat_r = np.asarray(feat_r, dtype=np.float32)
    nc = _get_nc()
    in_maps = _make_in_maps(feat_l, feat_r)
    res = run_bass_kernel_spmd(nc, in_maps, core_ids=list(range(NCORES)))
    return _decode(res.results, int(scale_x), int(scale_y))


def run_timed(np_inputs, trace_cores=None):
    """Run once with NTFF tracing enabled; returns exec_time_ns (or None)."""
    feat_l = np.asarray(np_inputs["feat_l"], dtype=np.float32)
    feat_r = np.asarray(np_inputs["feat_r"], dtype=np.float32)
    nc = _get_nc()
    in_maps = _make_in_maps(feat_l, feat_r)
    res = run_bass_kernel_spmd(nc, in_maps, core_ids=list(range(NCORES)),
                               trace=True, trace_cores=trace_cores)
    if res.instructions_and_trace is not None:
        print("trace path:", res.instructions_and_trace[1])
    return res.exec_time_ns


# revision 10
# speedup vs baseline: 1.2049x; 1.0941x over previous
"""Distributed volume-argmin (correlation volume max/argmax) on 8 NeuronCores.

Problem: feat_l, feat_r [1, 128, 96, 128] fp32.
  corr[b,h,w,ij] = sum_c feat_l[b,c,h,w] * feat_r[b,c,i,j]
  flow_cost = max over ij, flat = argmax over ij,
  flow = (xoff - (flat % W_R)*scale_x, yoff - (flat // W_R)*scale_y)

Sharding: the 96 left-image rows are split across 8 cores (12 rows = 1536
left pixels each); feat_r is replicated. Each core computes its
[1536 x 12288] correlation block (K=C=128 contraction on partitions) and a
full per-pixel max/argmax, so no cross-core reduction is needed — outputs
concatenate.

Matmul precision: inputs are split host-side into bf16 hi/lo pairs and each
512-chunk accumulates hi*hi + hi*lo + lo*hi in fp32 PSUM (fp32-grade
accuracy at 3x-bf16-pass cost).

Argmax strategy per 128-pixel tile (partition = left pixel, free = 12288
right pixels): a binary fold tree on the Vector engine
  colmax1[j] = max(x[j], x[j+6144])        (reads PSUM directly, drains it)
  colmax2[j] = max(colmax1[j], colmax1[j+3072])
  colmax3[j] = max(colmax2[j], colmax2[j+1536])
then MAX8 + FIND_INDEX8 on just 1536 elements give the exact max M and the
fold-residue index i*. The three fold bits are recovered exactly on the
Scalar engine: count_k = sum_j sign(M - upper_half_k[j]) equals the half
size minus 1 exactly when M lives in that upper half (sign(0)=0 on HW,
verified). flat = i* + 1536*c + 3072*b + 6144*a. The upper input half is
evicted PSUM->SBUF by ScalarE (needed as a fold operand and for count_1);
chunk groups are produced upper-half-first so every PSUM bank is drained
promptly.
"""

import sys

for p in ("/opt/trn_rl_repo",):
    if p not in sys.path:
        sys.path.insert(0, p)

import numpy as np
import ml_dtypes

import concourse.bass as bass
import concourse.tile as tile
from concourse import bacc, mybir
from concourse.bass_utils import run_bass_kernel_spmd

# Problem geometry (hardcoded per the task contract).
B, C, H, W = 1, 128, 96, 128
H_R, W_R = 96, 128
NPIX = H_R * W_R              # 12288 right pixels
NCORES = 8
ROWS_PER_CORE = H // NCORES   # 12 left rows per core
NT = ROWS_PER_CORE            # one tile per left row (128 pixels)
CHUNK = 512
GRP = 2048                    # psum group (4 banks)
NGRP = NPIX // GRP            # 6
WT = NPIX // 3               # third width: 4096 (ternary first level)
W2F = WT // 2                 # 2048 (fold2 output)
W3F = WT // 4                 # 1024 (fold3 output)

_F32 = mybir.dt.float32
_BF16 = mybir.dt.bfloat16
_U32 = mybir.dt.uint32
_FP8 = mybir.dt.float8e4

_compiled = {}


def _emit_group_mms(nc, ps_ap, lh, ll, rhs_hi, rhs_lo, g):
    """3-pass bf16 matmuls for the 4 chunks of group g into psum tile ps.

    Pass-major order (hi*hi x4, hi*lo x4, lo*hi x4) so consecutive matmuls
    write different PSUM banks — same-bank accumulation back-to-back
    serializes the PE's fill/drain pipeline (measured 379 ns vs 216 ns)."""
    for pass_lhs, pass_rhs, st, sp in (
        (lh, rhs_hi, True, False),
        (lh, rhs_lo, False, False),
        (ll, rhs_hi, False, True),
    ):
        for s in range(4):
            col0 = g * GRP + s * CHUNK
            out = ps_ap[:, s * CHUNK:(s + 1) * CHUNK]
            nc.tensor.matmul(out, pass_lhs, pass_rhs[:, col0:col0 + CHUNK],
                             start=st, stop=sp)


def _build_nc():
    nc = bacc.Bacc("TRN2", target_bir_lowering=False, debug=False,
                   num_devices=NCORES)

    lhs_hi_d = nc.declare_dram_parameter("lhs_hi", [C, NT * 128], _BF16,
                                         isOutput=False)
    lhs_lo_d = nc.declare_dram_parameter("lhs_lo", [C, NT * 128], _BF16,
                                         isOutput=False)
    rhs_hi_d = nc.declare_dram_parameter("rhs_hi", [C, NPIX], _BF16,
                                         isOutput=False)
    rhs_lo_d = nc.declare_dram_parameter("rhs_lo", [C, NPIX], _BF16,
                                         isOutput=False)
    cost_d = nc.declare_dram_parameter("cost", [128, NT], _F32, isOutput=True)
    idx_d = nc.declare_dram_parameter("idx", [128, NT], _U32, isOutput=True)
    cntu_d = nc.declare_dram_parameter("cntu", [128, NT], _F32, isOutput=True)
    cnta_d = nc.declare_dram_parameter("cnta", [128, NT], _F32, isOutput=True)
    cntb_d = nc.declare_dram_parameter("cntb", [128, NT], _F32, isOutput=True)
    cntc_d = nc.declare_dram_parameter("cntc", [128, NT], _F32, isOutput=True)

    with tile.TileContext(nc) as tc:
        with (
            tc.tile_pool(name="rhs", bufs=1) as rhs_pool,
            tc.tile_pool(name="lhs", bufs=1) as lhs_pool,
            tc.tile_pool(name="x2p", bufs=2) as x2_pool,
            tc.tile_pool(name="cmap", bufs=2) as cma_pool,
            tc.tile_pool(name="cm1p", bufs=2) as cm1_pool,
            tc.tile_pool(name="cm2p", bufs=1) as cm2_pool,
            tc.tile_pool(name="cm3p", bufs=2) as cm3_pool,
            tc.tile_pool(name="dumpp", bufs=1) as dump_pool,
            tc.tile_pool(name="m8p", bufs=3) as m8_pool,
            tc.tile_pool(name="ps", bufs=2, space="PSUM") as ps_pool,
            tc.tile_pool(name="outcols", bufs=1) as out_pool,
        ):
            lhs_hi = lhs_pool.tile([C, NT * 128], _BF16, tag="lh")
            lhs_lo = lhs_pool.tile([C, NT * 128], _BF16, tag="ll")
            rhs_hi = rhs_pool.tile([C, NPIX], _BF16, tag="rh")
            rhs_lo = rhs_pool.tile([C, NPIX], _BF16, tag="rl")

            # first matmuls (hi pass of group 4) need only lhs_hi + rhs_hi g4
            def dma_grp(t, d, g):
                sl = slice(g * GRP, (g + 1) * GRP)
                nc.sync.dma_start(t[:, sl], d[:, sl])

            dma_grp(rhs_hi, rhs_hi_d, 4)
            nc.sync.dma_start(lhs_hi[:], lhs_hi_d[:])
            dma_grp(rhs_lo, rhs_lo_d, 4)
            nc.sync.dma_start(lhs_lo[:], lhs_lo_d[:])
            for g in (5, 2, 3, 0, 1):
                dma_grp(rhs_hi, rhs_hi_d, g)
                dma_grp(rhs_lo, rhs_lo_d, g)

            cost_cols = out_pool.tile([128, NT], _F32, tag="cc")
            idx_cols = out_pool.tile([128, NT], _U32, tag="ic")
            cntu_cols = out_pool.tile([128, NT], _F32, tag="cu")
            cnta_cols = out_pool.tile([128, NT], _F32, tag="ca")
            cntb_cols = out_pool.tile([128, NT], _F32, tag="cb")
            cntc_cols = out_pool.tile([128, NT], _F32, tag="ccx")

            dump = dump_pool.tile([128, WT], _FP8, tag="dump")

            prev = None  # (x2, cma, cm1, cm2, m8, t) of previous tile

            def emit_counts(state):
                # small counts first: they unblock cm1/cm2 slab reuse
                x2_p, cma_p, cm1_p, cm2_p, m8_p, tp = state
                nc.scalar.activation(
                    dump[:, 0:W2F], cm1_p[:, W2F:WT],
                    mybir.ActivationFunctionType.Sign,
                    bias=m8_p[:, 0:1], scale=-1.0,
                    accum_out=cntb_cols[:, tp:tp + 1])
                nc.scalar.activation(
                    dump[:, 0:W3F], cm2_p[:, W3F:W2F],
                    mybir.ActivationFunctionType.Sign,
                    bias=m8_p[:, 0:1], scale=-1.0,
                    accum_out=cntc_cols[:, tp:tp + 1])
                nc.scalar.activation(
                    dump[:, 0:WT], x2_p[:],
                    mybir.ActivationFunctionType.Sign,
                    bias=m8_p[:, 0:1], scale=-1.0,
                    accum_out=cntu_cols[:, tp:tp + 1])
                nc.scalar.activation(
                    dump[:, 0:WT], cma_p[:],
                    mybir.ActivationFunctionType.Sign,
                    bias=m8_p[:, 0:1], scale=-1.0,
                    accum_out=cnta_cols[:, tp:tp + 1])

            for t in range(NT):
                lh = lhs_hi[:, t * 128:(t + 1) * 128]
                ll = lhs_lo[:, t * 128:(t + 1) * 128]

                x2 = x2_pool.tile([128, WT], _F32, tag="x2")
                cma = cma_pool.tile([128, WT], _F32, tag="cma")
                cm1 = cm1_pool.tile([128, WT], _F32, tag="cm1")

                # last third (cols 8192..12288 = groups 4,5): evict to x2
                for g in (4, 5):
                    ps = ps_pool.tile([128, GRP], _F32, tag="ps")
                    _emit_group_mms(nc, ps[:], lh, ll, rhs_hi[:], rhs_lo[:], g)
                    nc.scalar.copy(x2[:, (g - 4) * GRP:(g - 3) * GRP], ps[:])

                if prev is not None:
                    emit_counts(prev)

                # middle third (groups 2,3): fold against x2 -> cma
                for g in (2, 3):
                    ps = ps_pool.tile([128, GRP], _F32, tag="ps")
                    _emit_group_mms(nc, ps[:], lh, ll, rhs_hi[:], rhs_lo[:], g)
                    sl = slice((g - 2) * GRP, (g - 1) * GRP)
                    nc.vector.tensor_max(cma[:, sl], ps[:], x2[:, sl])

                # first third (groups 0,1): fold against cma -> cm1
                for g in (0, 1):
                    ps = ps_pool.tile([128, GRP], _F32, tag="ps")
                    _emit_group_mms(nc, ps[:], lh, ll, rhs_hi[:], rhs_lo[:], g)
                    sl = slice(g * GRP, (g + 1) * GRP)
                    nc.vector.tensor_max(cm1[:, sl], ps[:], cma[:, sl])

                cm2 = cm2_pool.tile([128, W2F], _F32, tag="cm2")
                nc.vector.tensor_max(cm2[:], cm1[:, 0:W2F], cm1[:, W2F:WT])
                cm3 = cm3_pool.tile([128, W3F], _F32, tag="cm3")
                nc.vector.tensor_max(cm3[:], cm2[:, 0:W3F], cm2[:, W3F:W2F])

                m8 = m8_pool.tile([128, 8], _F32, tag="m8")
                i8 = m8_pool.tile([128, 8], _U32, tag="i8")
                nc.vector.max(m8[:], cm3[:])
                nc.vector.max_index(i8[:], m8[:], cm3[:])
                nc.vector.tensor_copy(cost_cols[:, t:t + 1], m8[:, 0:1])
                nc.vector.tensor_copy(idx_cols[:, t:t + 1], i8[:, 0:1])

                prev = (x2, cma, cm1, cm2, m8, t)

            emit_counts(prev)

            nc.sync.dma_start(cost_d[:], cost_cols[:])
            nc.sync.dma_start(idx_d[:], idx_cols[:])
            nc.sync.dma_start(cntu_d[:], cntu_cols[:])
            nc.sync.dma_start(cnta_d[:], cnta_cols[:])
            nc.sync.dma_start(cntb_d[:], cntb_cols[:])
            nc.sync.dma_start(cntc_d[:], cntc_cols[:])

    nc.finalize()
    return nc


def _get_nc():
    if "nc" not in _compiled:
        _compiled["nc"] = _build_nc()
    return _compiled["nc"]


def _split_bf16(x):
    hi = x.astype(ml_dtypes.bfloat16)
    lo = (x - hi.astype(np.float32)).astype(ml_dtypes.bfloat16)
    return hi, lo


def _make_in_maps(feat_l, feat_r):
    rhs = np.ascontiguousarray(feat_r.reshape(C, NPIX))
    rhs_hi, rhs_lo = _split_bf16(rhs)
    in_maps = []
    for k in range(NCORES):
        lhs = np.ascontiguousarray(
            feat_l[0, :, k * ROWS_PER_CORE:(k + 1) * ROWS_PER_CORE, :]
            .reshape(C, ROWS_PER_CORE * W))
        lhs_hi, lhs_lo = _split_bf16(lhs)
        in_maps.append({
            "lhs_hi": lhs_hi, "lhs_lo": lhs_lo,
            "rhs_hi": rhs_hi, "rhs_lo": rhs_lo,
        })
    return in_maps


def _decode(results, scale_x, scale_y):
    flow_cost = np.empty((B, H, W), np.float32)
    flat = np.empty((H, W), np.int64)
    for k, r in enumerate(results):
        rows = slice(k * ROWS_PER_CORE, (k + 1) * ROWS_PER_CORE)
        flow_cost[0, rows, :] = r["cost"].T
        aU = (WT - r["cntu"].T.astype(np.int64))
        aA = (WT - r["cnta"].T.astype(np.int64))
        b = (W2F - r["cntb"].T.astype(np.int64))
        c = (W3F - r["cntc"].T.astype(np.int64))
        i_res = r["idx"].T.astype(np.int64)
        base = np.where(aU > 0, 2 * WT, np.where(aA > 0, WT, 0))
        flat[rows, :] = i_res + W3F * c + W2F * b + base
    assert flat.min() >= 0 and flat.max() < NPIX, (flat.min(), flat.max())
    xoff = np.arange(W)
    yoff = np.arange(H)
    u = -((flat % W_R) * scale_x - xoff[None, :]).astype(np.float32)
    v = -((flat // W_R) * scale_y - yoff[:, None]).astype(np.float32)
    flow = np.stack([u, v], axis=2)[None]  # [1, H, W, 2]
    return flow, flow_cost


def kernel(feat_l, feat_r, scale_x, scale_y):
    feat_l = np.asarray(feat_l, dtype=np.float32)
    feat_r = np.asarray(feat_r, dtype=np.float32)
    nc = _get_nc()
    in_maps = _make_in_maps(feat_l, feat_r)
    res = run_bass_kernel_spmd(nc, in_maps, core_ids=list(range(NCORES)))
    return _decode(res.results, int(scale_x), int(scale_y))


def run_timed(np_inputs, trace_cores=None):
    """Run once with NTFF tracing enabled; returns exec_time_ns (or None)."""
    feat_l = np.asarray(np_inputs["feat_l"], dtype=np.float32)
    feat_r = np.asarray(np_inputs["feat_r"], dtype=np.float32)
    nc = _get_nc()
    in_maps = _make_in_maps(feat_l, feat_r)
    res = run_bass_kernel_spmd(nc, in_maps, core_ids=list(range(NCORES)),
                               trace=True, trace_cores=trace_cores)
    if res.instructions_and_trace is not None:
        print("trace path:", res.instructions_and_trace[1])
    return res.exec_time_ns
